# revision 1
# baseline (speedup 1.0000x reference)
"""Trainium2 Bass kernel for nn_AttentionV4 (patch attention, 8 heads on 8 cores).

Pipeline per core (= per head h):
  - The 1x1 qkv conv + depthwise 3x3 conv are fused into one dense 3x3 conv,
    expressed as a single matmul over a 6x6-windowed patch basis:
      Q/K/V[r, n] = sum_kappa W4[kappa, r] * Xp[kappa, n],
    kappa = (ph, pw, c) in [6,6,48] (1728, chunked 18 x 96), n = interior
    patch (64x64 grid = 4096; boundary patches of the stride-4 pad-4 unfold
    are exactly zero and are handled analytically).
  - unfold/fold with kernel=stride=4 are pure reshapes; Xp windows are read
    straight out of a host-prepared block-layout image xb[hm,wm,c,hq,wq].
  - l2-normalize Q (x temperature) and K per column, A = Qn^T Kn in [-1,1],
    so softmax needs no max subtraction: E = exp(A), Z = rowsum(E) + 260
    (260 = number of zero boundary K columns, each contributing exp(0)).
  - out = (V/Z) @ E accumulated over 32 row-tiles of 128.
  - fold + AllToAll routes output stripe s to core s; each core applies the
    final 48x48 projection to its own 32-row stripe.
"""
import sys
import types

sys.path.insert(0, "/opt/trn_rl_repo")

import numpy as np

# ---------------------------------------------------------------- constants
C = 48          # image channels
CH = 6          # channels per head
NH = 8          # heads == cores
GN = 64         # interior patch grid
N = GN * GN     # 4096 interior patches
M96 = 96        # rows of a head matrix (6ch * 4 * 4)
NKAP = 1728     # 36 windows * 48 channels
NCHUNK = 18     # kappa chunks of 96
ZCORR = 260.0   # 4356 - 4096 zero K-columns, exp(0) each
NPIECE = 16     # front-end N pieces (4 patch rows, 256 patches each)
NCORES = 8

# chunk table: pairs of (hm, wm), (hm, wm+1) groups within one (dh, dw) class
def _chunk_table():
    info = []
    for dh, dw in [(0, 0), (0, 1), (1, 0), (1, 1)]:
        groups = [(hm, wm)
                  for hm in range(4 if dh == 0 else 2)
                  for wm in range(4 if dw == 0 else 2)]
        for i in range(0, len(groups), 2):
            (hm0, wm0), (hm1, wm1) = groups[i], groups[i + 1]
            assert hm0 == hm1 and wm1 == wm0 + 1
            info.append((dh, dw, hm0, wm0))
    assert len(info) == NCHUNK
    return info

CHUNKS = _chunk_table()

# group list (ph, pw) in kappa order
_GROUPS = []
for _dh, _dw in [(0, 0), (0, 1), (1, 0), (1, 1)]:
    for _hm in range(4 if _dh == 0 else 2):
        for _wm in range(4 if _dw == 0 else 2):
            _GROUPS.append((_dh * 4 + _hm, _dw * 4 + _wm))

# 14 chunks of K<=128; class-pure (class sizes are multiples of 128)
def _chunk_plan14():
    plan = []
    for k in range(14):
        k0, k1 = 128 * k, min(128 * (k + 1), NKAP)
        runs = []
        kap = k0
        dh = dw = None
        while kap < k1:
            g, c = divmod(kap, C)
            ph, pw = _GROUPS[g]
            if dh is None:
                dh, dw = ph // 4, pw // 4
            assert (ph // 4, pw // 4) == (dh, dw), "chunk crosses class"
            run_end = min((g + 1) * C, k1)
            runs.append((kap - k0, run_end - kap, ph % 4, pw % 4, c))
            kap = run_end
        plan.append((k1 - k0, dh, dw, runs))
    return plan

CHUNKS14 = _chunk_plan14()
NCHUNK14 = 14

# kappa order implied by the chunk table (ph, pw, c), c fastest
def _kappa_phpw():
    phs, pws = [], []
    for dh, dw, hm, wm0 in CHUNKS:
        for wm in (wm0, wm0 + 1):
            ph, pw = dh * 4 + hm, dw * 4 + wm
            phs += [ph] * C
            pws += [pw] * C
    return np.array(phs), np.array(pws)

_PHS, _PWS = _kappa_phpw()
_CS = np.tile(np.arange(C), NKAP // C)

# ---------------------------------------------------------------- host prep

def _build_xb(x):
    """Block layout of the (+1,+3)-padded image: xb[hm, wm, c, hq, wq]."""
    xpad = np.zeros((C, 260, 260), np.float32)
    xpad[:, 1:257, 1:257] = x[0]
    return np.ascontiguousarray(
        xpad.reshape(C, 65, 4, 65, 4).transpose(2, 4, 0, 1, 3))


def _build_w4(h, w_qkv, w_dw):
    """Fused (1x1 conv + dw3x3) weights in the kappa basis: [1728, 288]."""
    kh = np.arange(4)
    dy = _PHS[:, None] - kh[None, :]            # [1728, 4]
    dx = _PWS[:, None] - kh[None, :]
    my = (dy >= 0) & (dy < 3)
    mx = (dx >= 0) & (dx < 3)
    dyc = np.clip(dy, 0, 2)
    dxc = np.clip(dx, 0, 2)
    w4 = np.zeros((NKAP, 3, CH, 4, 4), np.float32)
    for sel in range(3):
        for cl in range(CH):
            o = sel * C + CH * h + cl
            wd = w_dw[o, 0]
            taps = (wd[dyc[:, :, None], dxc[:, None, :]]
                    * my[:, :, None] * mx[:, None, :])
            w4[:, sel, cl] = w_qkv[o, _CS][:, None, None] * taps
    return w4.reshape(NKAP, 288)


# ---------------------------------------------------------------- program

_PROG = None

def _build_program():
    import antenv  # noqa: F401
    if "antenv.axon_hooks" not in sys.modules:
        holder = {}
        m = types.ModuleType("antenv.axon_hooks")
        m.set_axon_ntff_profile_hook = lambda hk: holder.__setitem__("h", hk)
        m.get_axon_ntff_profile_hook = lambda: holder.get("h")
        sys.modules["antenv.axon_hooks"] = m
        antenv.axon_hooks = m
        try:
            from trn_agent_boot.trn_boot import _ntff_profile_via_ctypes
            m.set_axon_ntff_profile_hook(
                _ntff_profile_via_ctypes("/opt/axon/libaxon_pjrt.so"))
        except Exception:
            pass

    import concourse.bass as bass
    import concourse.tile as tile
    import concourse.mybir as mybir
    from contextlib import ExitStack

    F32 = mybir.dt.float32
    F32R = mybir.dt.float32r
    F16 = mybir.dt.float16
    AF = mybir.ActivationFunctionType

    nc = bass.Bass("TRN2", num_devices=NCORES)

    xb_h = nc.dram_tensor("xb", [4, 4, C, 65, 65], F32R, kind="ExternalInput")
    w4_h = nc.dram_tensor("w4", [NKAP, 288], F32R, kind="ExternalInput")
    vcol_h = nc.dram_tensor("vcol", [M96, 2], F32R, kind="ExternalInput")
    wpt_h = nc.dram_tensor("wpt", [C, C], F32R, kind="ExternalInput")
    id96_h = nc.dram_tensor("id96", [M96, M96], F32, kind="ExternalInput")
    ones_h = nc.dram_tensor("onesrow", [1, M96], F32, kind="ExternalInput")
    y_h = nc.dram_tensor("y", [C, 8192], F32, kind="ExternalOutput")
    cc_in = nc.dram_tensor("cc_in", [C, 8192], F32R)
    cc_out = nc.dram_tensor("cc_out", [C, 8192], F32R)

    with tile.TileContext(nc) as tc, ExitStack() as ctx, \
            nc.allow_low_precision(reason="float32r compute, fp32 tail"):
        const = ctx.enter_context(tc.tile_pool(name="const", bufs=1))
        w4_sb = const.tile([M96, NCHUNK * 288], F32R)
        nc.sync.dma_start(
            w4_sb[:].rearrange("p (k o) -> p k o", k=NCHUNK),
            w4_h[:].rearrange("(k p) o -> p k o", p=M96))
        vcol_sb = const.tile([M96, 2], F32R)
        nc.sync.dma_start(vcol_sb[:], vcol_h[:])
        wpt_sb = const.tile([C, C], F32R)
        nc.sync.dma_start(wpt_sb[:], wpt_h[:])
        id96_sb = const.tile([M96, M96], F32)
        nc.sync.dma_start(id96_sb[:], id96_h[:])
        ones_sb = const.tile([1, M96], F32)
        nc.sync.dma_start(ones_sb[:], ones_h[:])

        persist = ctx.enter_context(tc.tile_pool(name="persist", bufs=1))
        qn = persist.tile([M96, N], F32R)
        kn = persist.tile([M96, N], F32R)
        vt = persist.tile([128, 32 * M96], F32)
        zacc = persist.tile([128, 128], F32)
        rqt = persist.tile([128, 32], F32)

        # ---------------- front end: Q/K/V + column sumsq ----------------
        ctx2 = tc.tile_pool(name="fe_persist", bufs=1)
        fep = ctx2.__enter__()
        vn = fep.tile([M96, N], F32)
        rq_row = fep.tile([1, N], F32)
        rk_row = fep.tile([1, N], F32)
        ph_ps_cm = tc.tile_pool(name="ph1_ps", bufs=1, space="PSUM")
        ph1ps = ph_ps_cm.__enter__()
        with tc.tile_pool(name="fe_xp", bufs=4) as xp_pool, \
             tc.tile_pool(name="fe_tmp", bufs=2) as fe_tmp:
            for pp in range(NPIECE // 2):
                xps = []
                for half in range(2):
                    p = 2 * pp + half
                    r0 = 4 * p
                    xp_t = xp_pool.tile([M96, NCHUNK, 5, 65], F32R,
                                        name=f"xp{half}", tag="xp")
                    for k, (dh, dw, hm, wm0) in enumerate(CHUNKS):
                        nc.sync.dma_start(
                            xp_t[:, k, :, :],
                            xb_h[hm, wm0:wm0 + 2, :, r0:r0 + 5, :]
                            .rearrange("a c r w -> (a c) r w"))
                    xps.append(xp_t)
                for sel, dst in ((0, qn), (1, kn), (2, vn)):
                    pss = [ph1ps.tile([M96, 256], F32, name=f"ps{half}",
                                      tag="ps", bufs=3)
                           for half in range(2)]
                    for k, (dh, dw, hm, wm0) in enumerate(CHUNKS):
                        for half in range(2):
                            nc.tensor.matmul(
                                pss[half][:],
                                lhsT=w4_sb[:, 288 * k + M96 * sel:
                                           288 * k + M96 * (sel + 1)],
                                rhs=xps[half][:, k, dh:dh + 4, dw:dw + 64],
                                start=(k == 0), stop=(k == NCHUNK - 1))
                    for half in range(2):
                        p = 2 * pp + half
                        cols = slice(256 * p, 256 * (p + 1))
                        nc.vector.tensor_copy(dst[:, cols], pss[half][:])
                        if sel < 2:
                            sq = fe_tmp.tile([M96, 256], F32R, name=f"sq{half}",
                                             tag="sq")
                            nc.scalar.activation(sq[:], pss[half][:], AF.Square)
                            ssp = ph1ps.tile([1, 256], F32, name=f"ssp{half}",
                                             tag="ssp", bufs=2)
                            nc.tensor.matmul(
                                ssp[:], lhsT=vcol_sb[:, sel:sel + 1], rhs=sq[:],
                                start=True, stop=True)
                            row = rq_row if sel == 0 else rk_row
                            nc.vector.tensor_copy(row[0:1, cols], ssp[:])

        # ---------------- rqt = rsqrt(sumsq_q) in row-tile layout ----------------
        if True:
            rqps = ph1ps.tile([128, 32], F32, bufs=1)
            for t in range(32):
                nc.tensor.transpose(
                    rqps[:, t:t + 1], rq_row[0:1, 128 * t:128 * (t + 1)],
                    ones_sb[0:1, 0:1])
            nc.vector.reciprocal(rqt[:], rqps[:])
            nc.scalar.activation(rqt[:], rqt[:], AF.Sqrt)

        # ---------------- normalize K ----------------
        with tc.tile_pool(name="nrm", bufs=2) as npool:
            for mt in range(8):
                cols = slice(512 * mt, 512 * (mt + 1))
                bp = ph1ps.tile([M96, 512], F32, name="bp", tag="bp", bufs=1)
                nc.tensor.matmul(bp[:], lhsT=ones_sb[:], rhs=rk_row[0:1, cols],
                                 start=True, stop=True)
                b = npool.tile([M96, 512], F32)
                nc.vector.reciprocal(b[:], bp[:])
                nc.scalar.activation(b[:], b[:], AF.Sqrt)
                nc.vector.tensor_mul(kn[:, cols], kn[:, cols], b[:])

        # ---------------- V^T via PE transpose ----------------
        if True:
            for t in range(32):
                tp = ph1ps.tile([128, M96], F32, name="tp", tag="tp", bufs=1)
                nc.tensor.transpose(
                    tp[:], vn[:, 128 * t:128 * (t + 1)], id96_sb[:])
                nc.vector.tensor_copy(vt[:, M96 * t:M96 * (t + 1)], tp[:])
        ph_ps_cm.__exit__(None, None, None)
        ctx2.__exit__(None, None, None)
        late = ctx.enter_context(tc.tile_pool(name="late", bufs=1))
        out_acc = late.tile([M96, N], F32)

        # ---------------- attention ----------------
        with tc.tile_pool(name="a_ps", bufs=3, space="PSUM") as apsum, \
             tc.tile_pool(name="o_ps", bufs=2, space="PSUM") as opsum, \
             tc.tile_pool(name="e_sb", bufs=6) as epool, \
             tc.tile_pool(name="z_sb", bufs=2) as zpool, \
             tc.tile_pool(name="vts", bufs=8) as vtspool:
            for g in range(8):
                estrips = []
                for tl in range(4):
                    t = 4 * g + tl
                    es = epool.tile([128, N], F32R)
                    estrips.append(es)
                    for mp in range(4):
                        pa = apsum.tile([128, 1024], F32)
                        for half in range(2):
                            nc.tensor.matmul(
                                pa[:, 512 * half:512 * (half + 1)],
                                lhsT=qn[:, 128 * t:128 * (t + 1)],
                                rhs=kn[:, 1024 * mp + 512 * half:
                                       1024 * mp + 512 * (half + 1)],
                                start=True, stop=True)
                        col = 4 * t + mp
                        nc.scalar.activation(
                            es[:, 1024 * mp:1024 * (mp + 1)], pa[:], AF.Exp,
                            scale=rqt[:, t:t + 1],
                            accum_out=zacc[:, col:col + 1])
                # Z for the group's 4 row-tiles: sum 4 accum cols, +260, 1/x
                zinv = zpool.tile([128, 4], F32)
                nc.vector.tensor_reduce(
                    zinv[:],
                    zacc[:, 16 * g:16 * (g + 1)].rearrange(
                        "p (t m) -> p t m", t=4),
                    axis=mybir.AxisListType.X, op=mybir.AluOpType.add)
                nc.vector.tensor_scalar_add(zinv[:], zinv[:], ZCORR)
                nc.vector.reciprocal(zinv[:], zinv[:])
                vts_tiles = []
                for tl in range(4):
                    t = 4 * g + tl
                    vts = vtspool.tile([128, M96], F32R)
                    nc.vector.tensor_scalar_mul(
                        vts[:], vt[:, M96 * t:M96 * (t + 1)],
                        zinv[:, tl:tl + 1])
                    vts_tiles.append(vts)
                for jpair in range(4):
                    for sub in range(2):
                        pos = [opsum.tile([M96, 512], F32, tag="pos",
                                          name=f"pos{jj}")
                               for jj in range(2)]
                        for tl in (2 * sub, 2 * sub + 1):
                            for jj in range(2):
                                j = 2 * jpair + jj
                                nc.tensor.matmul(
                                    pos[jj][:], lhsT=vts_tiles[tl],
                                    rhs=estrips[tl][:, 512 * j:512 * (j + 1)],
                                    start=(tl == 2 * sub),
                                    stop=(tl == 2 * sub + 1))
                        for jj in range(2):
                            j = 2 * jpair + jj
                            cols = slice(512 * j, 512 * (j + 1))
                            if g == 0 and sub == 0:
                                nc.vector.tensor_copy(
                                    out_acc[:, cols], pos[jj][:])
                            else:
                                nc.vector.tensor_add(
                                    out_acc[:, cols], out_acc[:, cols],
                                    pos[jj][:])

        # ---------------- fold staging + AllToAll + projection ----------------
        prl_cm = tc.tile_pool(name="prl", bufs=1)
        prl = prl_cm.__enter__()
        out_acc_r = prl.tile([M96, N], F32R)
        nc.vector.tensor_copy(out_acc_r[:], out_acc[:])
        nc.sync.dma_start(
            cc_in[:].rearrange("(s cl) (khw i j) -> (cl khw) s i j",
                               s=8, cl=CH, khw=16, i=8),
            out_acc_r[:].rearrange("p (s i j) -> p s i j", s=8, i=8))
        nc.gpsimd.collective_compute(
            "AllToAll", mybir.AluOpType.bypass,
            replica_groups=[list(range(NCORES))],
            ins=[cc_in[:]], outs=[cc_out[:]])
        with tc.tile_pool(name="prj", bufs=2) as prj, \
             tc.tile_pool(name="prj_ps", bufs=2, space="PSUM") as prjps, \
             tc.tile_pool(name="yt", bufs=2) as ypool:
            for q in range(16):
                cols = slice(512 * q, 512 * (q + 1))
                fold_t = prj.tile([C, 512], F32R)
                nc.sync.dma_start(fold_t[:], cc_out[:, cols])
                pp = prjps.tile([C, 512], F32)
                nc.tensor.matmul(pp[:], lhsT=wpt_sb[:], rhs=fold_t[:],
                                 start=True, stop=True)
                yt = ypool.tile([C, 512], F32)
                nc.vector.tensor_copy(yt[:], pp[:])
                nc.sync.dma_start(y_h[:, cols], yt[:])
        prl_cm.__exit__(None, None, None)

    _split_excess_waits(nc)
    return nc


_wsplit_ctr = [0]

def _split_excess_waits(nc, max_waits=1):
    """This walrus build encodes only one sync-wait per instruction; hoist
    extras onto same-engine nops inserted directly before the instruction."""
    import bass_rust
    import concourse.mybir as mybir
    for fn in nc.m.functions:
        for bb in fn.blocks:
            insts = bb.instructions
            out = []
            changed = False
            for inst in insts:
                si = inst.sync_info
                if si is not None and len(si.on_wait) > max_waits:
                    waits = list(si.on_wait)
                    for w in waits[:-max_waits]:
                        _wsplit_ctr[0] += 1
                        nop = bass_rust.InstNoOp(
                            name=f"I-wsplit-{_wsplit_ctr[0]}", ins=[], outs=[])
                        nop.engine = inst.engine
                        nop.sync_info = mybir.SyncInfo(
                            on_wait=[w], on_update=[])
                        out.append(nop)
                    inst.sync_info = mybir.SyncInfo(
                        on_wait=waits[-max_waits:],
                        on_update=list(si.on_update))
                    changed = True
                out.append(inst)
            if changed:
                bb.instructions = out


def _get_program():
    global _PROG
    if _PROG is None:
        _PROG = _build_program()
    return _PROG


# ---------------------------------------------------------------- entry

def kernel(x, w_qkv, w_dw, temperature, w_proj, _trace=False):
    x = np.asarray(x, np.float32)
    w_qkv = np.asarray(w_qkv, np.float32)
    w_dw = np.asarray(w_dw, np.float32)
    temperature = np.asarray(temperature, np.float32)
    w_proj = np.asarray(w_proj, np.float32)

    nc = _get_program()
    from concourse.bass_utils import run_bass_kernel_spmd

    xb = _build_xb(x)
    id96 = np.eye(M96, dtype=np.float32)
    wpt = np.ascontiguousarray(w_proj.T)
    in_maps = []
    for h in range(NH):
        t_h = float(temperature[h, 0, 0])
        vcol = np.empty((M96, 2), np.float32)
        vcol[:, 0] = 1.0 / (t_h * t_h)
        vcol[:, 1] = 1.0
        in_maps.append({
            "xb": xb,
            "w4": _build_w4(h, w_qkv, w_dw),
            "vcol": vcol,
            "wpt": wpt,
            "id96": id96,
            "onesrow": np.ones((1, M96), np.float32),
        })

    res = run_bass_kernel_spmd(nc, in_maps, list(range(NCORES)), trace=_trace)

    y = np.empty((1, C, 256, 256), np.float32)
    for s in range(NCORES):
        blk = res.results[s]["y"].reshape(C, 4, 4, 8, GN)
        y[0, :, 32 * s:32 * (s + 1), :] = (
            blk.transpose(0, 3, 1, 4, 2).reshape(C, 32, 256))
    if _trace:
        return y, res
    return y



# revision 3
# speedup vs baseline: 1.4354x; 1.4354x over previous
"""Trainium2 Bass kernel for nn_AttentionV4 (patch attention, 8 heads on 8 cores).

Pipeline per core (= per head h), v2:
  - The 1x1 qkv conv + depthwise 3x3 conv are fused into one dense matmul over
    a 6x6-windowed patch basis (kappa = (ph, pw, c) in [6,6,48] = 1728,
    chunked 14 x 128), n = interior patch (64x64 grid = 4096; boundary patches
    of the stride-4 pad-4 unfold are exactly zero, handled analytically).
  - All matmul data in bf16 (tolerance 2e-2 gives plenty of headroom); PSUM
    accumulation stays fp32.
  - The windowed rhs (xp) is pre-gathered on the host into a dense per-piece
    layout so each piece is ONE big DMA (the old per-chunk gather serialized
    the sync queue).
  - Column norms of Q/K and V^T transposes are fused into the front-end piece
    loop (no serial normalize phase).
  - Attention: A = Q^T K in [-1,1]*temp scaled per-row by rqt inside the exp;
    E = exp in bf16; Z = rowsum via ACT accumulators (+260 for the zero
    boundary K columns); out = (V/Z) @ E accumulated 4 row-tiles per PSUM
    group, software-pipelined one group ahead of the exp stream.
  - Tail: no collective. Each core applies its head's slice of the final 48x48
    projection (arranged per pixel-phase) to its own full-image output and the
    host sums the 8 partial projections.
"""
import sys
import types

sys.path.insert(0, "/opt/trn_rl_repo")

import numpy as np
import ml_dtypes

BF = ml_dtypes.bfloat16

# ---------------------------------------------------------------- constants
C = 48          # image channels
CH = 6          # channels per head
NH = 8          # heads == cores
GN = 64         # interior patch grid
N = GN * GN     # 4096 interior patches
M96 = 96        # rows of a head matrix (6ch * 4 * 4)
NKAP = 1728     # 36 windows * 48 channels
ZCORR = 260.0   # 4356 - 4096 zero K-columns, exp(0) each
NPIECE = 16     # front-end N pieces (4 patch rows, 256 patches each)
NCORES = 8
NCH14 = 14      # kappa chunks of <=128

# group list (ph, pw) in kappa order: class-major ((dh,dw) in order), then
# hm-major, wm-minor inside the class
_GROUPS = []
for _dh, _dw in [(0, 0), (0, 1), (1, 0), (1, 1)]:
    for _hm in range(4 if _dh == 0 else 2):
        for _wm in range(4 if _dw == 0 else 2):
            _GROUPS.append((_dh * 4 + _hm, _dw * 4 + _wm))

_PHS = np.array([g[0] for g in _GROUPS for _ in range(C)])
_PWS = np.array([g[1] for g in _GROUPS for _ in range(C)])
_CS = np.tile(np.arange(C), NKAP // C)


def _chunk_plan14():
    """14 chunks of K<=128; class-pure (class sizes are multiples of 128).

    Each entry: (krows, dh, dw, runs) with runs = (off, len, hm, wm, c0).
    """
    plan = []
    for k in range(NCH14):
        k0, k1 = 128 * k, min(128 * (k + 1), NKAP)
        runs = []
        kap = k0
        dh = dw = None
        while kap < k1:
            g, c = divmod(kap, C)
            ph, pw = _GROUPS[g]
            if dh is None:
                dh, dw = ph // 4, pw // 4
            assert (ph // 4, pw // 4) == (dh, dw), "chunk crosses class"
            run_end = min((g + 1) * C, k1)
            runs.append((kap - k0, run_end - kap, ph % 4, pw % 4, c))
            kap = run_end
        plan.append((k1 - k0, dh, dw, runs))
    return plan


CHUNKS14 = _chunk_plan14()

# ---------------------------------------------------------------- host prep


def _build_xb(x):
    """Block layout of the (+1,+3)-padded image: xb[hm, wm, c, hq, wq]."""
    xpad = np.zeros((C, 260, 260), np.float32)
    xpad[:, 1:257, 1:257] = x[0]
    return np.ascontiguousarray(
        xpad.reshape(C, 65, 4, 65, 4).transpose(2, 4, 0, 1, 3))


def _build_xp(x):
    """Per-piece dense windowed rhs: xp[piece, 128, 14, 5, 65] bf16."""
    xb = _build_xb(x)           # [4, 4, C, 65, 65]
    xp = np.zeros((NPIECE, 128, NCH14, 5, 65), np.float32)
    for p in range(NPIECE):
        r0 = 4 * p
        rows = min(5, 65 - r0)
        for k, (krows, dh, dw, runs) in enumerate(CHUNKS14):
            for (off, ln, hm, wm, c0) in runs:
                xp[p, off:off + ln, k, :rows, :] = \
                    xb[hm, wm, c0:c0 + ln, r0:r0 + rows, :]
    return np.ascontiguousarray(xp.astype(BF))


def _build_w4(h, w_qkv, w_dw):
    """Fused (1x1 conv + dw3x3) weights in the kappa basis: [128,14,288]."""
    kh = np.arange(4)
    dy = _PHS[:, None] - kh[None, :]            # [1728, 4]
    dx = _PWS[:, None] - kh[None, :]
    my = (dy >= 0) & (dy < 3)
    mx = (dx >= 0) & (dx < 3)
    dyc = np.clip(dy, 0, 2)
    dxc = np.clip(dx, 0, 2)
    w4 = np.zeros((NKAP, 3, CH, 4, 4), np.float32)
    for sel in range(3):
        for cl in range(CH):
            o = sel * C + CH * h + cl
            wd = w_dw[o, 0]
            taps = (wd[dyc[:, :, None], dxc[:, None, :]]
                    * my[:, :, None] * mx[:, None, :])
            w4[:, sel, cl] = w_qkv[o, _CS][:, None, None] * taps
    w4 = w4.reshape(NKAP, 288)
    w4p = np.zeros((128, NCH14, 288), np.float32)
    for k in range(NCH14):
        k0, k1 = 128 * k, min(128 * (k + 1), NKAP)
        w4p[:k1 - k0, k, :] = w4[k0:k1]
    return np.ascontiguousarray(w4p.astype(BF))


def _build_wproj16(h, w_proj):
    """Per-head phase-blocked projection: [96, 8*96] bf16.

    Column block pp2 holds output rows (o + 48*pl) for phases p = 2*pp2 + pl,
    p = 4*kh + kw.  lhsT[(cl,kh,kw), 96*pp2 + 48*pl + o] = w_proj[o, 6h+cl]
    iff (kh,kw) matches phase p.
    """
    w16 = np.zeros((M96, 8 * M96), np.float32)
    for p in range(16):
        kh, kw = p // 4, p % 4
        pp2, pl = p // 2, p % 2
        rows = np.arange(CH) * 16 + 4 * kh + kw          # (cl, kh, kw)
        cols = 96 * pp2 + 48 * pl + np.arange(C)         # (o)
        w16[np.ix_(rows, cols)] = w_proj[:, CH * h:CH * h + CH].T
    return np.ascontiguousarray(w16.astype(BF))


# ---------------------------------------------------------------- program

_PROG = None


def _build_program():
    import antenv  # noqa: F401
    if "antenv.axon_hooks" not in sys.modules:
        holder = {}
        m = types.ModuleType("antenv.axon_hooks")
        m.set_axon_ntff_profile_hook = lambda hk: holder.__setitem__("h", hk)
        m.get_axon_ntff_profile_hook = lambda: holder.get("h")
        sys.modules["antenv.axon_hooks"] = m
        antenv.axon_hooks = m
        try:
            from trn_agent_boot.trn_boot import _ntff_profile_via_ctypes
            m.set_axon_ntff_profile_hook(
                _ntff_profile_via_ctypes("/opt/axon/libaxon_pjrt.so"))
        except Exception:
            pass

    import concourse.bass as bass
    import concourse.tile as tile
    import concourse.mybir as mybir
    from contextlib import ExitStack

    F32 = mybir.dt.float32
    BF16 = mybir.dt.bfloat16
    AF = mybir.ActivationFunctionType

    nc = bass.Bass("TRN2", num_devices=NCORES)

    xp_h = nc.dram_tensor("xp", [NPIECE, 128, NCH14, 5, 65], BF16,
                          kind="ExternalInput")
    w4_h = nc.dram_tensor("w4", [128, NCH14, 288], BF16, kind="ExternalInput")
    vcol_h = nc.dram_tensor("vcol", [M96, 2], BF16, kind="ExternalInput")
    w16_h = nc.dram_tensor("w16", [M96, 8 * M96], BF16, kind="ExternalInput")
    id96_h = nc.dram_tensor("id96", [M96, M96], BF16, kind="ExternalInput")
    onesb_h = nc.dram_tensor("onesb", [1, M96], BF16, kind="ExternalInput")
    onesf_h = nc.dram_tensor("onesf", [1, M96], F32, kind="ExternalInput")
    y_h = nc.dram_tensor("y", [8, M96, N], BF16, kind="ExternalOutput")

    with tile.TileContext(nc) as tc, ExitStack() as ctx, \
            nc.allow_low_precision(reason="bf16 compute, fp32 accum"):
        const = ctx.enter_context(tc.tile_pool(name="const", bufs=1))
        w4_sb = const.tile([128, NCH14, 288], BF16)
        nc.sync.dma_start(w4_sb[:], w4_h[:])
        vcol_sb = const.tile([M96, 2], BF16)
        nc.sync.dma_start(vcol_sb[:], vcol_h[:])
        w16_sb = const.tile([M96, 8 * M96], BF16)
        nc.sync.dma_start(w16_sb[:], w16_h[:])
        id96_sb = const.tile([M96, M96], BF16)
        nc.sync.dma_start(id96_sb[:], id96_h[:])
        onesb_sb = const.tile([1, M96], BF16)
        nc.sync.dma_start(onesb_sb[:], onesb_h[:])
        onesf_sb = const.tile([1, M96], F32)
        nc.sync.dma_start(onesf_sb[:], onesf_h[:])

        persist = ctx.enter_context(tc.tile_pool(name="persist", bufs=1))
        qn = persist.tile([M96, N], BF16)
        kn = persist.tile([M96, N], BF16)
        vt = persist.tile([128, 32 * M96], BF16)
        rqt = persist.tile([128, 32], F32)
        zacc = persist.tile([128, 96], F32)
        rq_row = persist.tile([1, N], F32)
        out_acc = persist.tile([M96, N], F32)
        oa_b = persist.tile([M96, N], BF16)

        # ---------------- front end: Q/K/V + norms + V^T, per piece --------
        fe_cm = ExitStack()
        xp_pool = fe_cm.enter_context(tc.tile_pool(name="fe_xp", bufs=3))
        sq_pool = fe_cm.enter_context(tc.tile_pool(name="fe_sq", bufs=2))
        vn_pool = fe_cm.enter_context(tc.tile_pool(name="fe_vn", bufs=2))
        row_pool = fe_cm.enter_context(tc.tile_pool(name="fe_row", bufs=4))
        bkn_pool = fe_cm.enter_context(tc.tile_pool(name="fe_bkn", bufs=2))
        feps = fe_cm.enter_context(
            tc.tile_pool(name="fe_ps", bufs=1, space="PSUM"))
        rqps = feps.tile([128, 32], F32, bufs=1)
        for p in range(NPIECE):
            cols = slice(256 * p, 256 * (p + 1))
            xp_t = xp_pool.tile([128, NCH14, 5, 65], BF16, name="xp", tag="xp")
            nc.sync.dma_start(xp_t[:], xp_h[p])
            for sel in range(3):
                ps = feps.tile([M96, 256], F32, name="ps", tag="ps", bufs=3)
                for k, (krows, dh, dw, runs) in enumerate(CHUNKS14):
                    nc.tensor.matmul(
                        ps[:],
                        lhsT=w4_sb[:, k, M96 * sel:M96 * (sel + 1)],
                        rhs=xp_t[:, k, dh:dh + 4, dw:dw + 64],
                        start=(k == 0), stop=(k == NCH14 - 1))
                if sel == 0:      # Q: raw copy + sumsq row (scaled 1/t^2)
                    sq = sq_pool.tile([M96, 256], BF16, name="sq", tag="sq")
                    nc.scalar.activation(sq[:], ps[:], AF.Square)
                    ssp = feps.tile([1, 256], F32, name="ssp", tag="ssp",
                                    bufs=2)
                    nc.tensor.matmul(ssp[:], lhsT=vcol_sb[:, 0:1], rhs=sq[:],
                                     start=True, stop=True)
                    nc.scalar.activation(rq_row[0:1, cols], ssp[:], AF.Copy)
                    nc.vector.tensor_copy(qn[:, cols], ps[:])
                elif sel == 1:    # K: sumsq -> rsqrt -> broadcast -> scale
                    sq = sq_pool.tile([M96, 256], BF16, name="sq", tag="sq")
                    nc.scalar.activation(sq[:], ps[:], AF.Square)
                    ssp = feps.tile([1, 256], F32, name="ssp", tag="ssp",
                                    bufs=2)
                    nc.tensor.matmul(ssp[:], lhsT=vcol_sb[:, 1:2], rhs=sq[:],
                                     start=True, stop=True)
                    rk_f = row_pool.tile([1, 256], F32, name="rkf", tag="rkf")
                    nc.vector.reciprocal(rk_f[:], ssp[:])
                    rkn = row_pool.tile([1, 256], BF16, name="rkn", tag="rkn")
                    nc.scalar.activation(rkn[:], rk_f[:], AF.Sqrt)
                    bkps = feps.tile([M96, 256], F32, name="bkps", tag="bkps",
                                     bufs=1)
                    nc.tensor.matmul(bkps[:], lhsT=onesb_sb[0:1, 0:M96],
                                     rhs=rkn[:], start=True, stop=True)
                    bkn = bkn_pool.tile([M96, 256], F32, name="bkn", tag="bkn")
                    nc.scalar.activation(bkn[:], bkps[:], AF.Copy)
                    nc.vector.tensor_mul(kn[:, cols], ps[:], bkn[:])
                else:             # V: copy + transpose both 128-col halves
                    vn_p = vn_pool.tile([M96, 256], BF16, name="vn", tag="vn")
                    nc.vector.tensor_copy(vn_p[:], ps[:])
                    for half in range(2):
                        ti = 2 * p + half
                        tp = feps.tile([128, M96], BF16, name="tp", tag="tp",
                                       bufs=1)
                        nc.tensor.transpose(
                            tp[:], vn_p[:, 128 * half:128 * (half + 1)],
                            id96_sb[:])
                        nc.vector.tensor_copy(
                            vt[:, M96 * ti:M96 * (ti + 1)], tp[:])
        # rqt = rsqrt(sumsq_q) in [128, 32] row-tile layout
        for t in range(32):
            nc.tensor.transpose(
                rqps[:, t:t + 1], rq_row[0:1, 128 * t:128 * (t + 1)],
                onesf_sb[0:1, 0:1])
        nc.vector.reciprocal(rqt[:], rqps[:])
        nc.scalar.activation(rqt[:], rqt[:], AF.Sqrt)
        fe_cm.close()

        # ---------------- attention, pipelined one group ahead ------------
        at_cm = ExitStack()
        papool = at_cm.enter_context(
            tc.tile_pool(name="a_pa", bufs=1, space="PSUM"))
        pbpool = at_cm.enter_context(
            tc.tile_pool(name="a_pb", bufs=1, space="PSUM"))
        opool = at_cm.enter_context(
            tc.tile_pool(name="a_op", bufs=2, space="PSUM"))
        espool = at_cm.enter_context(tc.tile_pool(name="a_es", bufs=8))
        vtspool = at_cm.enter_context(tc.tile_pool(name="a_vts", bufs=8))
        zpool = at_cm.enter_context(tc.tile_pool(name="a_z", bufs=2))

        es_tiles = {}
        vts_tiles = {}

        def emit_out(g2, j0, nj):
            for j in range(j0, j0 + nj):
                cj = slice(512 * j, 512 * (j + 1))
                op = opool.tile([M96, 512], F32, name="op", tag="op")
                for tl2 in range(4):
                    t2 = 4 * g2 + tl2
                    nc.tensor.matmul(
                        op[:], lhsT=vts_tiles[t2],
                        rhs=es_tiles[t2][:, cj],
                        start=(tl2 == 0), stop=(tl2 == 3))
                if g2 == 0:
                    nc.vector.tensor_copy(out_acc[:, cj], op[:])
                else:
                    nc.vector.tensor_add(out_acc[:, cj], out_acc[:, cj],
                                         op[:])
                if g2 == 7:
                    nc.vector.tensor_copy(oa_b[:, cj], out_acc[:, cj])

        MSPLITS = ((0, 2048), (2048, 1024), (3072, 1024))
        for g in range(8):
            for tl in range(4):
                t = 4 * g + tl
                es = espool.tile([128, N], BF16, name="es", tag="es")
                es_tiles[t] = es
                for si, (m0, mw) in enumerate(MSPLITS):
                    pool = papool if si == 0 else pbpool
                    pa = pool.tile([128, mw], F32, name=f"pa{si}", tag="pa")
                    for i in range(mw // 512):
                        nc.tensor.matmul(
                            pa[:, 512 * i:512 * (i + 1)],
                            lhsT=qn[:, 128 * t:128 * (t + 1)],
                            rhs=kn[:, m0 + 512 * i:m0 + 512 * (i + 1)],
                            start=True, stop=True)
                    nc.scalar.activation(
                        es[:, m0:m0 + mw], pa[:], AF.Exp,
                        scale=rqt[:, t:t + 1],
                        accum_out=zacc[:, 3 * t + si:3 * t + si + 1])
                    if si == 0 and g > 0:
                        emit_out(g - 1, 2 * tl, 2)
            # group epilogue: zinv + vts
            zs = zpool.tile([128, 4], F32, name="zs", tag="zs")
            nc.vector.tensor_reduce(
                zs[:],
                zacc[:, 12 * g:12 * (g + 1)].rearrange(
                    "p (t x) -> p t x", t=4),
                axis=mybir.AxisListType.X, op=mybir.AluOpType.add)
            nc.vector.tensor_scalar_add(zs[:], zs[:], ZCORR)
            nc.vector.reciprocal(zs[:], zs[:])
            for tl in range(4):
                t = 4 * g + tl
                vts = vtspool.tile([128, M96], BF16, name="vts", tag="vts")
                nc.vector.tensor_scalar_mul(
                    vts[:], vt[:, M96 * t:M96 * (t + 1)], zs[:, tl:tl + 1])
                vts_tiles[t] = vts
        emit_out(7, 0, 8)
        at_cm.close()

        # ---------------- per-head partial projection tail ----------------
        with tc.tile_pool(name="prj_ps", bufs=4, space="PSUM") as prjps, \
                tc.tile_pool(name="yt", bufs=2) as ypool:
            for pp2 in range(8):
                yt = ypool.tile([M96, N], BF16, name="yt", tag="yt")
                for jn in range(8):
                    cj = slice(512 * jn, 512 * (jn + 1))
                    pp_ps = prjps.tile([M96, 512], F32, name="pps", tag="pps")
                    nc.tensor.matmul(
                        pp_ps[:], lhsT=w16_sb[:, M96 * pp2:M96 * (pp2 + 1)],
                        rhs=oa_b[:, cj], start=True, stop=True)
                    if jn % 2 == 0:
                        nc.vector.tensor_copy(yt[:, cj], pp_ps[:])
                    else:
                        nc.scalar.activation(yt[:, cj], pp_ps[:], AF.Copy)
                nc.sync.dma_start(y_h[pp2], yt[:])

    _split_excess_waits(nc)
    return nc


_wsplit_ctr = [0]


def _split_excess_waits(nc, max_waits=1):
    """This walrus build encodes only one sync-wait per instruction; hoist
    extras onto same-engine nops inserted directly before the instruction."""
    import bass_rust
    import concourse.mybir as mybir
    for fn in nc.m.functions:
        for bb in fn.blocks:
            insts = bb.instructions
            out = []
            changed = False
            for inst in insts:
                si = inst.sync_info
                if si is not None and len(si.on_wait) > max_waits:
                    waits = list(si.on_wait)
                    for w in waits[:-max_waits]:
                        _wsplit_ctr[0] += 1
                        nop = bass_rust.InstNoOp(
                            name=f"I-wsplit-{_wsplit_ctr[0]}", ins=[], outs=[])
                        nop.engine = inst.engine
                        nop.sync_info = mybir.SyncInfo(
                            on_wait=[w], on_update=[])
                        out.append(nop)
                    inst.sync_info = mybir.SyncInfo(
                        on_wait=waits[-max_waits:],
                        on_update=list(si.on_update))
                    changed = True
                out.append(inst)
            if changed:
                bb.instructions = out


def _get_program():
    global _PROG
    if _PROG is None:
        _PROG = _build_program()
    return _PROG


# ---------------------------------------------------------------- entry

def kernel(x, w_qkv, w_dw, temperature, w_proj, _trace=False):
    x = np.asarray(x, np.float32)
    w_qkv = np.asarray(w_qkv, np.float32)
    w_dw = np.asarray(w_dw, np.float32)
    temperature = np.asarray(temperature, np.float32)
    w_proj = np.asarray(w_proj, np.float32)

    nc = _get_program()
    from concourse.bass_utils import run_bass_kernel_spmd

    xp = _build_xp(x)
    id96 = np.eye(M96, dtype=BF)
    onesb = np.ones((1, M96), BF)
    onesf = np.ones((1, M96), np.float32)
    in_maps = []
    for h in range(NH):
        t_h = float(temperature[h, 0, 0])
        vcol = np.empty((M96, 2), np.float32)
        vcol[:, 0] = 1.0 / (t_h * t_h)
        vcol[:, 1] = 1.0
        in_maps.append({
            "xp": xp,
            "w4": _build_w4(h, w_qkv, w_dw),
            "vcol": vcol.astype(BF),
            "w16": _build_wproj16(h, w_proj),
            "id96": id96,
            "onesb": onesb,
            "onesf": onesf,
        })

    res = run_bass_kernel_spmd(nc, in_maps, list(range(NCORES)), trace=_trace)

    # host gather: sum per-head partial projections, then phase reassembly
    acc = np.zeros((8, M96, N), np.float32)
    for s in range(NCORES):
        acc += np.asarray(res.results[s]["y"]).astype(np.float32)
    # rows: [pp2, (pl, o)], phase p = 2*pp2 + pl = 4*kh + kw
    acc = acc.reshape(8, 2, C, GN, GN).reshape(16, C, GN, GN)
    y = np.ascontiguousarray(
        acc.reshape(4, 4, C, GN, GN).transpose(2, 3, 0, 4, 1)
        .reshape(C, 256, 256))[None]
    if _trace:
        return y, res
    return y


# revision 10
# speedup vs baseline: 1.4499x; 1.0101x over previous
"""Trainium2 Bass kernel for nn_AttentionV4 (patch attention, 8 heads on 8 cores).

Pipeline per core (= per head h), v2:
  - The 1x1 qkv conv + depthwise 3x3 conv are fused into one dense matmul over
    a 6x6-windowed patch basis (kappa = (ph, pw, c) in [6,6,48] = 1728,
    chunked 14 x 128), n = interior patch (64x64 grid = 4096; boundary patches
    of the stride-4 pad-4 unfold are exactly zero, handled analytically).
  - All matmul data in bf16 (tolerance 2e-2 gives plenty of headroom); PSUM
    accumulation stays fp32.
  - The windowed rhs (xp) is pre-gathered on the host into a dense per-piece
    layout so each piece is ONE big DMA (the old per-chunk gather serialized
    the sync queue).
  - Column norms of Q/K and V^T transposes are fused into the front-end piece
    loop (no serial normalize phase).
  - Attention: A = Q^T K in [-1,1]*temp scaled per-row by rqt inside the exp;
    E = exp in bf16; Z = rowsum via ACT accumulators (+260 for the zero
    boundary K columns); out = (V/Z) @ E accumulated 4 row-tiles per PSUM
    group, software-pipelined one group ahead of the exp stream.
  - Tail: no collective. Each core applies its head's slice of the final 48x48
    projection (arranged per pixel-phase) to its own full-image output and the
    host sums the 8 partial projections.
"""
import sys
import types

sys.path.insert(0, "/opt/trn_rl_repo")

import numpy as np
import ml_dtypes

BF = ml_dtypes.bfloat16

# ---------------------------------------------------------------- constants
C = 48          # image channels
CH = 6          # channels per head
NH = 8          # heads == cores
GN = 64         # interior patch grid
N = GN * GN     # 4096 interior patches
M96 = 96        # rows of a head matrix (6ch * 4 * 4)
NKAP = 1728     # 36 windows * 48 channels
ZCORR = 260.0   # 4356 - 4096 zero K-columns, exp(0) each
NPIECE = 8      # front-end N pieces (8 patch rows, 512 patches each)
NCORES = 8
NCH14 = 14      # kappa chunks of <=128

# group list (ph, pw) in kappa order: class-major ((dh,dw) in order), then
# hm-major, wm-minor inside the class
_GROUPS = []
for _dh, _dw in [(0, 0), (0, 1), (1, 0), (1, 1)]:
    for _hm in range(4 if _dh == 0 else 2):
        for _wm in range(4 if _dw == 0 else 2):
            _GROUPS.append((_dh * 4 + _hm, _dw * 4 + _wm))

_PHS = np.array([g[0] for g in _GROUPS for _ in range(C)])
_PWS = np.array([g[1] for g in _GROUPS for _ in range(C)])
_CS = np.tile(np.arange(C), NKAP // C)


def _chunk_plan14():
    """14 chunks of K<=128; class-pure (class sizes are multiples of 128).

    Each entry: (krows, dh, dw, runs) with runs = (off, len, hm, wm, c0).
    """
    plan = []
    for k in range(NCH14):
        k0, k1 = 128 * k, min(128 * (k + 1), NKAP)
        runs = []
        kap = k0
        dh = dw = None
        while kap < k1:
            g, c = divmod(kap, C)
            ph, pw = _GROUPS[g]
            if dh is None:
                dh, dw = ph // 4, pw // 4
            assert (ph // 4, pw // 4) == (dh, dw), "chunk crosses class"
            run_end = min((g + 1) * C, k1)
            runs.append((kap - k0, run_end - kap, ph % 4, pw % 4, c))
            kap = run_end
        plan.append((k1 - k0, dh, dw, runs))
    return plan


CHUNKS14 = _chunk_plan14()

# ---------------------------------------------------------------- host prep


def _build_xb(x):
    """Block layout of the (+1,+3)-padded image: xb[hm, wm, c, hq, wq]."""
    xpad = np.zeros((C, 260, 260), np.float32)
    xpad[:, 1:257, 1:257] = x[0]
    return np.ascontiguousarray(
        xpad.reshape(C, 65, 4, 65, 4).transpose(2, 4, 0, 1, 3))


def _build_xp(x):
    """Per-piece dense windowed rhs: xp[piece, 128, 14, 9, 65] bf16."""
    xb = _build_xb(x)           # [4, 4, C, 65, 65]
    xp = np.zeros((NPIECE, 128, NCH14, 9, 65), np.float32)
    for p in range(NPIECE):
        r0 = 8 * p
        for k, (krows, dh, dw, runs) in enumerate(CHUNKS14):
            for (off, ln, hm, wm, c0) in runs:
                xp[p, off:off + ln, k, :, :] = \
                    xb[hm, wm, c0:c0 + ln, r0:r0 + 9, :]
    return np.ascontiguousarray(xp.astype(BF))


def _build_w4(h, w_qkv, w_dw):
    """Fused (1x1 conv + dw3x3) weights in the kappa basis: [128,14,288]."""
    kh = np.arange(4)
    dy = _PHS[:, None] - kh[None, :]            # [1728, 4]
    dx = _PWS[:, None] - kh[None, :]
    my = (dy >= 0) & (dy < 3)
    mx = (dx >= 0) & (dx < 3)
    dyc = np.clip(dy, 0, 2)
    dxc = np.clip(dx, 0, 2)
    w4 = np.zeros((NKAP, 3, CH, 4, 4), np.float32)
    for sel in range(3):
        for cl in range(CH):
            o = sel * C + CH * h + cl
            wd = w_dw[o, 0]
            taps = (wd[dyc[:, :, None], dxc[:, None, :]]
                    * my[:, :, None] * mx[:, None, :])
            w4[:, sel, cl] = w_qkv[o, _CS][:, None, None] * taps
    w4 = w4.reshape(NKAP, 288)
    w4p = np.zeros((128, NCH14, 288), np.float32)
    for k in range(NCH14):
        k0, k1 = 128 * k, min(128 * (k + 1), NKAP)
        w4p[:k1 - k0, k, :] = w4[k0:k1]
    return np.ascontiguousarray(w4p.astype(BF))


def _build_wproj16(h, w_proj):
    """Per-head phase-blocked projection: [96, 8*96] bf16.

    Column block pp2 holds output rows (o + 48*pl) for phases p = 2*pp2 + pl,
    p = 4*kh + kw.  lhsT[(cl,kh,kw), 96*pp2 + 48*pl + o] = w_proj[o, 6h+cl]
    iff (kh,kw) matches phase p.
    """
    w16 = np.zeros((M96, 8 * M96), np.float32)
    for p in range(16):
        kh, kw = p // 4, p % 4
        pp2, pl = p // 2, p % 2
        rows = np.arange(CH) * 16 + 4 * kh + kw          # (cl, kh, kw)
        cols = 96 * pp2 + 48 * pl + np.arange(C)         # (o)
        w16[np.ix_(rows, cols)] = w_proj[:, CH * h:CH * h + CH].T
    return np.ascontiguousarray(w16.astype(BF))


# ---------------------------------------------------------------- program

_PROG = None


def _build_program():
    import antenv  # noqa: F401
    if "antenv.axon_hooks" not in sys.modules:
        holder = {}
        m = types.ModuleType("antenv.axon_hooks")
        m.set_axon_ntff_profile_hook = lambda hk: holder.__setitem__("h", hk)
        m.get_axon_ntff_profile_hook = lambda: holder.get("h")
        sys.modules["antenv.axon_hooks"] = m
        antenv.axon_hooks = m
        try:
            from trn_agent_boot.trn_boot import _ntff_profile_via_ctypes
            m.set_axon_ntff_profile_hook(
                _ntff_profile_via_ctypes("/opt/axon/libaxon_pjrt.so"))
        except Exception:
            pass

    import concourse.bass as bass
    import concourse.tile as tile
    import concourse.mybir as mybir
    from contextlib import ExitStack

    F32 = mybir.dt.float32
    BF16 = mybir.dt.bfloat16
    AF = mybir.ActivationFunctionType

    nc = bass.Bass("TRN2", num_devices=NCORES)

    xp_h = nc.dram_tensor("xp", [NPIECE, 128, NCH14, 9, 65], BF16,
                          kind="ExternalInput")
    w4_h = nc.dram_tensor("w4", [128, NCH14, 288], BF16, kind="ExternalInput")
    vcol_h = nc.dram_tensor("vcol", [M96, 2], BF16, kind="ExternalInput")
    w16_h = nc.dram_tensor("w16", [M96, 8 * M96], BF16, kind="ExternalInput")
    id96_h = nc.dram_tensor("id96", [M96, M96], BF16, kind="ExternalInput")
    onesb_h = nc.dram_tensor("onesb", [1, M96], BF16, kind="ExternalInput")
    onesf_h = nc.dram_tensor("onesf", [1, M96], F32, kind="ExternalInput")
    y_h = nc.dram_tensor("y", [8, M96, N], BF16, kind="ExternalOutput")

    with tile.TileContext(nc) as tc, ExitStack() as ctx, \
            nc.allow_low_precision(reason="bf16 compute, fp32 accum"):
        const = ctx.enter_context(tc.tile_pool(name="const", bufs=1))
        w4_sb = const.tile([128, NCH14, 288], BF16)
        nc.sync.dma_start(w4_sb[:], w4_h[:])
        vcol_sb = const.tile([M96, 2], BF16)
        nc.sync.dma_start(vcol_sb[:], vcol_h[:])
        w16_sb = const.tile([M96, 8 * M96], BF16)
        nc.sync.dma_start(w16_sb[:], w16_h[:])
        id96_sb = const.tile([M96, M96], BF16)
        nc.sync.dma_start(id96_sb[:], id96_h[:])
        onesb_sb = const.tile([1, M96], BF16)
        nc.sync.dma_start(onesb_sb[:], onesb_h[:])
        onesf_sb = const.tile([1, M96], F32)
        nc.sync.dma_start(onesf_sb[:], onesf_h[:])

        persist = ctx.enter_context(tc.tile_pool(name="persist", bufs=1))
        qn = persist.tile([M96, N], BF16)
        kn = persist.tile([M96, N], BF16)
        vt = persist.tile([128, 32 * M96], BF16)
        rqt = persist.tile([128, 32], F32)
        zacc = persist.tile([128, 96], F32)
        rq_row = persist.tile([1, N], F32)
        out_acc = persist.tile([M96, N], F32)
        oa_b = persist.tile([M96, N], BF16)

        # ---------------- front end: Q/K/V + norms + V^T, per piece --------
        fe_cm = ExitStack()
        xp_pool = fe_cm.enter_context(tc.tile_pool(name="fe_xp", bufs=3))
        sq_pool = fe_cm.enter_context(tc.tile_pool(name="fe_sq", bufs=2))
        vn_pool = fe_cm.enter_context(tc.tile_pool(name="fe_vn", bufs=2))
        row_pool = fe_cm.enter_context(tc.tile_pool(name="fe_row", bufs=4))
        bkn_pool = fe_cm.enter_context(tc.tile_pool(name="fe_bkn", bufs=2))
        feps = fe_cm.enter_context(
            tc.tile_pool(name="fe_ps", bufs=1, space="PSUM"))
        rqps = feps.tile([128, 32], F32, bufs=1)
        for p in range(NPIECE):
            cols = slice(512 * p, 512 * (p + 1))
            xp_t = xp_pool.tile([128, NCH14, 9, 65], BF16, name="xp", tag="xp")
            nc.sync.dma_start(xp_t[:], xp_h[p])
            for sel in range(3):
                ps = feps.tile([M96, 512], F32, name="ps", tag="ps", bufs=3)
                for k, (krows, dh, dw, runs) in enumerate(CHUNKS14):
                    nc.tensor.matmul(
                        ps[:],
                        lhsT=w4_sb[:, k, M96 * sel:M96 * (sel + 1)],
                        rhs=xp_t[:, k, dh:dh + 8, dw:dw + 64],
                        start=(k == 0), stop=(k == NCH14 - 1))
                if sel == 0:      # Q: raw copy + sumsq row (scaled 1/t^2)
                    sq = sq_pool.tile([M96, 512], BF16, name="sq", tag="sq")
                    nc.scalar.activation(sq[:], ps[:], AF.Square)
                    ssp = feps.tile([1, 512], F32, name="ssp", tag="ssp",
                                    bufs=2)
                    nc.tensor.matmul(ssp[:], lhsT=vcol_sb[:, 0:1], rhs=sq[:],
                                     start=True, stop=True)
                    nc.scalar.activation(rq_row[0:1, cols], ssp[:], AF.Copy)
                    nc.vector.tensor_copy(qn[:, cols], ps[:])
                elif sel == 1:    # K: sumsq -> bcast -> rsqrt -> scale
                    sq = sq_pool.tile([M96, 512], BF16, name="sq", tag="sq")
                    nc.scalar.activation(sq[:], ps[:], AF.Square)
                    ssp = feps.tile([1, 512], F32, name="ssp", tag="ssp",
                                    bufs=2)
                    nc.tensor.matmul(ssp[:], lhsT=vcol_sb[:, 1:2], rhs=sq[:],
                                     start=True, stop=True)
                    ssb = row_pool.tile([1, 512], BF16, name="ssb", tag="ssb")
                    nc.scalar.activation(ssb[:], ssp[:], AF.Copy)
                    bkps = feps.tile([M96, 512], F32, name="bkps", tag="bkps",
                                     bufs=1)
                    nc.tensor.matmul(bkps[:], lhsT=onesb_sb[0:1, 0:M96],
                                     rhs=ssb[:], start=True, stop=True)
                    rrec = bkn_pool.tile([M96, 512], F32, name="rrec",
                                         tag="rrec")
                    nc.vector.reciprocal(rrec[:], bkps[:])
                    bkn = bkn_pool.tile([M96, 512], F32, name="bkn", tag="bkn")
                    nc.scalar.activation(bkn[:], rrec[:], AF.Sqrt)
                    nc.vector.tensor_mul(kn[:, cols], ps[:], bkn[:])
                else:             # V: copy + transpose the 128-col quarters
                    vn_p = vn_pool.tile([M96, 512], BF16, name="vn", tag="vn")
                    nc.vector.tensor_copy(vn_p[:], ps[:])
                    for half in range(4):
                        ti = 4 * p + half
                        tp = feps.tile([128, M96], BF16, name="tp", tag="tp",
                                       bufs=1)
                        nc.tensor.transpose(
                            tp[:], vn_p[:, 128 * half:128 * (half + 1)],
                            id96_sb[:])
                        nc.vector.tensor_copy(
                            vt[:, M96 * ti:M96 * (ti + 1)], tp[:])
        # rqt = rsqrt(sumsq_q) in [128, 32] row-tile layout
        for t in range(32):
            nc.tensor.transpose(
                rqps[:, t:t + 1], rq_row[0:1, 128 * t:128 * (t + 1)],
                onesf_sb[0:1, 0:1])
        nc.vector.reciprocal(rqt[:], rqps[:])
        nc.scalar.activation(rqt[:], rqt[:], AF.Sqrt)
        fe_cm.close()

        # ---------------- attention, pipelined one group ahead ------------
        at_cm = ExitStack()
        papool = at_cm.enter_context(
            tc.tile_pool(name="a_pa", bufs=1, space="PSUM"))
        pbpool = at_cm.enter_context(
            tc.tile_pool(name="a_pb", bufs=1, space="PSUM"))
        opool = at_cm.enter_context(
            tc.tile_pool(name="a_op", bufs=2, space="PSUM"))
        espool = at_cm.enter_context(tc.tile_pool(name="a_es", bufs=8))
        vtspool = at_cm.enter_context(tc.tile_pool(name="a_vts", bufs=8))
        zpool = at_cm.enter_context(tc.tile_pool(name="a_z", bufs=2))

        es_tiles = {}
        vts_tiles = {}

        def emit_out(g2, j0, nj):
            for j in range(j0, j0 + nj):
                cj = slice(512 * j, 512 * (j + 1))
                op = opool.tile([M96, 512], F32, name="op", tag="op")
                for tl2 in range(4):
                    t2 = 4 * g2 + tl2
                    nc.tensor.matmul(
                        op[:], lhsT=vts_tiles[t2],
                        rhs=es_tiles[t2][:, cj],
                        start=(tl2 == 0), stop=(tl2 == 3))
                if g2 == 0:
                    nc.vector.tensor_copy(out_acc[:, cj], op[:])
                else:
                    nc.vector.tensor_add(out_acc[:, cj], out_acc[:, cj],
                                         op[:])
                if g2 == 7:
                    nc.vector.tensor_copy(oa_b[:, cj], out_acc[:, cj])

        MSPLITS = ((0, 2048), (2048, 1024), (3072, 1024))
        for g in range(8):
            for tl in range(4):
                t = 4 * g + tl
                es = espool.tile([128, N], BF16, name="es", tag="es")
                es_tiles[t] = es
                for si, (m0, mw) in enumerate(MSPLITS):
                    pool = papool if si == 0 else pbpool
                    pa = pool.tile([128, mw], F32, name=f"pa{si}", tag="pa")
                    for i in range(mw // 512):
                        nc.tensor.matmul(
                            pa[:, 512 * i:512 * (i + 1)],
                            lhsT=qn[:, 128 * t:128 * (t + 1)],
                            rhs=kn[:, m0 + 512 * i:m0 + 512 * (i + 1)],
                            start=True, stop=True)
                    nc.scalar.activation(
                        es[:, m0:m0 + mw], pa[:], AF.Exp,
                        scale=rqt[:, t:t + 1],
                        accum_out=zacc[:, 3 * t + si:3 * t + si + 1])
                if g > 0:
                    emit_out(g - 1, 2 * tl, 2)
            # group epilogue: zinv + vts
            zs = zpool.tile([128, 4], F32, name="zs", tag="zs")
            nc.vector.tensor_reduce(
                zs[:],
                zacc[:, 12 * g:12 * (g + 1)].rearrange(
                    "p (t x) -> p t x", t=4),
                axis=mybir.AxisListType.X, op=mybir.AluOpType.add)
            nc.vector.tensor_scalar_add(zs[:], zs[:], ZCORR)
            nc.vector.reciprocal(zs[:], zs[:])
            for tl in range(4):
                t = 4 * g + tl
                vts = vtspool.tile([128, M96], BF16, name="vts", tag="vts")
                nc.vector.tensor_scalar_mul(
                    vts[:], vt[:, M96 * t:M96 * (t + 1)], zs[:, tl:tl + 1])
                vts_tiles[t] = vts
        emit_out(7, 0, 8)
        at_cm.close()

        # ---------------- per-head partial projection tail ----------------
        with tc.tile_pool(name="prj_ps", bufs=4, space="PSUM") as prjps, \
                tc.tile_pool(name="yt", bufs=2) as ypool:
            for jn in range(8):
                cj = slice(512 * jn, 512 * (jn + 1))
                yt = ypool.tile([M96, N], BF16, name="yt", tag="yt")
                for pp2 in range(8):
                    cb = slice(512 * pp2, 512 * (pp2 + 1))
                    pp_ps = prjps.tile([M96, 512], F32, name="pps", tag="pps")
                    nc.tensor.matmul(
                        pp_ps[:], lhsT=w16_sb[:, M96 * pp2:M96 * (pp2 + 1)],
                        rhs=oa_b[:, cj], start=True, stop=True)
                    if pp2 % 2 == 0:
                        nc.vector.tensor_copy(yt[:, cb], pp_ps[:])
                    else:
                        nc.scalar.activation(yt[:, cb], pp_ps[:], AF.Copy)
                nc.sync.dma_start(y_h[jn], yt[:])

    _split_excess_waits(nc)
    return nc


_wsplit_ctr = [0]


def _split_excess_waits(nc, max_waits=1):
    """This walrus build encodes only one sync-wait per instruction; hoist
    extras onto same-engine nops inserted directly before the instruction."""
    import bass_rust
    import concourse.mybir as mybir
    for fn in nc.m.functions:
        for bb in fn.blocks:
            insts = bb.instructions
            out = []
            changed = False
            for inst in insts:
                si = inst.sync_info
                if si is not None and len(si.on_wait) > max_waits:
                    waits = list(si.on_wait)
                    for w in waits[:-max_waits]:
                        _wsplit_ctr[0] += 1
                        nop = bass_rust.InstNoOp(
                            name=f"I-wsplit-{_wsplit_ctr[0]}", ins=[], outs=[])
                        nop.engine = inst.engine
                        nop.sync_info = mybir.SyncInfo(
                            on_wait=[w], on_update=[])
                        out.append(nop)
                    inst.sync_info = mybir.SyncInfo(
                        on_wait=waits[-max_waits:],
                        on_update=list(si.on_update))
                    changed = True
                out.append(inst)
            if changed:
                bb.instructions = out


def _get_program():
    global _PROG
    if _PROG is None:
        _PROG = _build_program()
    return _PROG


# ---------------------------------------------------------------- entry

def kernel(x, w_qkv, w_dw, temperature, w_proj, _trace=False):
    x = np.asarray(x, np.float32)
    w_qkv = np.asarray(w_qkv, np.float32)
    w_dw = np.asarray(w_dw, np.float32)
    temperature = np.asarray(temperature, np.float32)
    w_proj = np.asarray(w_proj, np.float32)

    nc = _get_program()
    from concourse.bass_utils import run_bass_kernel_spmd

    xp = _build_xp(x)
    id96 = np.eye(M96, dtype=BF)
    onesb = np.ones((1, M96), BF)
    onesf = np.ones((1, M96), np.float32)
    in_maps = []
    for h in range(NH):
        t_h = float(temperature[h, 0, 0])
        vcol = np.empty((M96, 2), np.float32)
        vcol[:, 0] = 1.0 / (t_h * t_h)
        vcol[:, 1] = 1.0
        in_maps.append({
            "xp": xp,
            "w4": _build_w4(h, w_qkv, w_dw),
            "vcol": vcol.astype(BF),
            "w16": _build_wproj16(h, w_proj),
            "id96": id96,
            "onesb": onesb,
            "onesf": onesf,
        })

    res = run_bass_kernel_spmd(nc, in_maps, list(range(NCORES)), trace=_trace)

    # host gather: sum per-head partial projections, then phase reassembly
    acc = np.zeros((8, M96, N), np.float32)
    for s in range(NCORES):
        acc += np.asarray(res.results[s]["y"]).astype(np.float32)
    # acc[jn, (pl, o), (pp2, i)] -> [p = (pp2, pl), o, n = (jn, i)]
    acc = acc.reshape(8, 2, C, 8, 512).transpose(3, 1, 2, 0, 4)
    acc = acc.reshape(16, C, GN, GN)
    y = np.ascontiguousarray(
        acc.reshape(4, 4, C, GN, GN).transpose(2, 3, 0, 4, 1)
        .reshape(C, 256, 256))[None]
    if _trace:
        return y, res
    return y
